# revision 32
# baseline (speedup 1.0000x reference)
"""Distributed multi-head attention kernel for one TRN2 chip (8 NeuronCores).

Problem: x[2,2048,1024] -> qkv -> 16-head attention -> out proj, f32 I/O.

Sharding: 8 cores = 2 batches x 4 head-groups (4 heads each).
Core c: batch b=c//4, head group g=c%4 (heads 4g..4g+3).

Redesign vs the 330us baseline (all measured on HW micro-benches):
 - Score matmuls (K=64) issued ALTERNATING PE row-groups (0,0)/(64,0):
   consecutive MMs stream concurrently through disjoint array halves
   (131ns/MM vs 426 serial).
 - PV matmuls in fp8e4 DoubleRow: two k-chunks per MM (2x PV throughput),
   ones-column trick still emits softmax denominators (row 64).
 - Softmax exp on ScalarE in [128,4096] instructions from an SBUF bf16
   stage (DVE evicts psum): (4096+352)/1.2 = 0.90ns/elem vs 1.15 at 1024.
 - qkT phase-1 (pair-0 q,k) k-OUTER across 8 psum banks so matmuls start
   while x is still streaming in; pair-1 qkT + v production interleaved
   into the first two attention units.
 - Weights packed host-side into wide-line DRAM tensors (16KB/partition
   lines) to avoid the descriptor-bound input DMA of the baseline; biases
   DMA'd first; ACT table warm-up at t=0.
 - Per-(pair,qtile) AllGathers (8 small ones) + flipped projection
   (outT = wp.T @ oT, N=512) streamed 2 units behind the collectives;
   f32 output written transposed, host transposes back.
"""

import os
import sys
import types
import numpy as np
import ml_dtypes

import concourse.bass as bass
import concourse.mybir as mybir
import concourse.bacc as bacc
import concourse.tile as tile
from concourse.bass_utils import run_bass_kernel_spmd

BF16 = mybir.dt.bfloat16
F32 = mybir.dt.float32
FP8 = mybir.dt.float8e4

B, N, D = 2, 2048, 1024
H, DH = 16, 64
SCALE = DH ** -0.5

P = 128                  # partitions
NT = 512                 # token tile (qtile width, matmul N)
KC = N // P              # 16 k-token chunks
QT = N // NT             # 4 q tiles
DC = D // P              # 8 d_model chunks
HPC = 4                  # heads per core
OF = HPC * DH            # 256 o-features per core
VH = 68                  # v8 per-head stride: [v(64) | ones | pad3]
VKO = HPC * VH           # 272 (ko stride in v8, %16==0 for DoubleRow)

CORE_IDS = list(range(8))
GROUPS = [[0, 1, 2, 3], [4, 5, 6, 7]]
# attention unit order: pair-0 units early (only need qkT phase G1);
# first p1 unit at slot 3 (after G2 finishes inside slot 2), and the
# per-qt AllGathers spread across slots 4..8.
UNITS = [(0, 0), (0, 1), (0, 2), (1, 0), (0, 3), (1, 1), (1, 2), (1, 3)]
LAST_RESULTS = None


def _install_ntff_shim():
    if "antenv.axon_hooks" in sys.modules:
        return
    try:
        from trn_agent_boot.trn_boot import _ntff_profile_via_ctypes
        hook = _ntff_profile_via_ctypes("/opt/axon/libaxon_pjrt.so")
    except Exception:
        hook = None
    mod = types.ModuleType("antenv.axon_hooks")
    mod._hook = hook
    mod.get_axon_ntff_profile_hook = lambda: mod._hook
    mod.set_axon_ntff_profile_hook = lambda h: setattr(mod, "_hook", h)
    sys.modules["antenv.axon_hooks"] = mod


def build_nc():
    nc = bacc.Bacc("TRN2", target_bir_lowering=False, debug=False, num_devices=8)

    xt_ext = nc.dram_tensor("xt", [D, N], BF16, kind="ExternalInput")
    wqkp_ext = nc.dram_tensor("wqkp", [P, DC * 512], BF16, kind="ExternalInput")
    wvpp_ext = nc.dram_tensor("wvpp", [P, DC * 512], BF16, kind="ExternalInput")
    bqk_ext = nc.dram_tensor("bqk", [P, 4], F32, kind="ExternalInput")
    bv_ext = nc.dram_tensor("bv", [1, OF], F32, kind="ExternalInput")
    bp_ext = nc.dram_tensor("bp", [P, 2], F32, kind="ExternalInput")
    outT_ext = nc.dram_tensor("outT", [OF, N], F32, kind="ExternalOutput")

    ag_in = {}
    ag_out = {}
    for qt in range(3):
        ag_in[qt] = nc.dram_tensor(f"agi{qt}", [2 * P, NT], BF16)
        ag_out[qt] = nc.dram_tensor(f"ago{qt}", [8 * P, NT], BF16)
    for p in range(2):  # qt3 split per pair so only the p1 gather is exposed
        ag_in[(3, p)] = nc.dram_tensor(f"agi3{p}", [P, NT], BF16)
        ag_out[(3, p)] = nc.dram_tensor(f"ago3{p}", [4 * P, NT], BF16)

    with tile.TileContext(nc) as tc:
        with (
            tc.tile_pool(name="const_pool", bufs=1) as const_pool,
            tc.tile_pool(name="xt_pool", bufs=1) as xt_pool,
            tc.tile_pool(name="w_pool", bufs=1) as w_pool,
            tc.tile_pool(name="qk_pool", bufs=1) as qk_pool,
            tc.tile_pool(name="v8_pool", bufs=1) as v8_pool,
        ):
            # ---- tiny tensors first (unblock ACT + evictions early) ----
            bqk_sb = const_pool.tile([P, 4], F32)
            nc.sync.dma_start(bqk_sb[:], bqk_ext[:])
            bv_row = const_pool.tile([1, OF], F32)
            nc.sync.dma_start(bv_row[:], bv_ext[:])
            bp_sb = const_pool.tile([P, 2], F32)
            nc.sync.dma_start(bp_sb[:], bp_ext[:])

            # ACT table warm-up: get the 1.5us Exp ACT_TABLE_LOAD done at t=0
            wu_in = const_pool.tile([P, 16], BF16)
            nc.vector.memset(wu_in[:], 0.0)
            wu_out = const_pool.tile([P, 16], FP8)
            nc.scalar.activation(wu_out[:], wu_in[:],
                                 mybir.ActivationFunctionType.Exp)

            bv_bc = const_pool.tile([P, OF], F32)
            nc.gpsimd.partition_broadcast(bv_bc[:], bv_row[:])
            ones64 = const_pool.tile([1, 64], F32)
            nc.vector.memset(ones64[:], 1.0)

            # ---- bulk input DMA (wide lines, interleaved for early start) --
            wqkp = w_pool.tile([P, DC * 512], BF16, name="wqkp")
            wvpp = w_pool.tile([P, DC * 512], BF16, name="wvpp")
            xt_t = [xt_pool.tile([P, N], BF16, name=f"xt{k}") for k in range(DC)]

            def dma4(dst, src, splits=4):
                # split one logical load over several DMA rings
                step = P // splits
                for q in range(splits):
                    nc.sync.dma_start(dst[q * step:(q + 1) * step, :],
                                      src[q * step:(q + 1) * step, :])

            dma4(wqkp[:, 0:2048], wqkp_ext[:, 0:2048])
            for k in range(3):
                dma4(xt_t[k][:], xt_ext[k * P:(k + 1) * P, :])
            dma4(wqkp[:, 2048:4096], wqkp_ext[:, 2048:4096])
            for k in range(3, DC):
                dma4(xt_t[k][:], xt_ext[k * P:(k + 1) * P, :])
            dma4(wvpp[:], wvpp_ext[:])

            def wqk_s(k, m):
                return wqkp[:, k * 512 + m * P:k * 512 + (m + 1) * P]

            def wv_s(k):
                return wvpp[:, k * OF:(k + 1) * OF]

            def wp_s(kk, f):
                return wvpp[:, 2048 + kk * OF + f * P:2048 + kk * OF + (f + 1) * P]

            # qkT output: m=0: q heads 0-1, 1: q heads 2-3, 2: k heads 0-1,
            # 3: k heads 2-3  (q side pre-scaled by 1/sqrt(dh) on host)
            qk_sb = [qk_pool.tile([P, N], BF16, name=f"qk{m}") for m in range(4)]
            # v8[j]: fp8 DoubleRow stationary for chunk pair j (chunks 2j,2j+1)
            # layout [ko(2) x (head(4) x [v(64)|1|pad3])]
            v8 = [v8_pool.tile([P, 2 * VKO], FP8, name=f"v8{j}")
                  for j in range(KC // 2)]

            # ---- G1: qkT for m in {0,2}; n-outer so (m,n=0) evicts first
            # and the pair-0 attention pipeline starts ASAP ----
            with tc.tile_pool(name="psB", bufs=4, space="PSUM") as psB:
                for n in range(QT):
                    for m in (0, 2):
                        t = psB.tile([P, NT], F32, name="g1")
                        for k in range(DC):
                            nc.tensor.matmul(
                                t[:], wqk_s(k, m),
                                xt_t[k][:, n * NT:(n + 1) * NT],
                                start=(k == 0), stop=(k == DC - 1))
                        nc.vector.tensor_scalar_add(
                            qk_sb[m][:, n * NT:(n + 1) * NT],
                            t[:], bqk_sb[:, m:m + 1])

            # ---- attention + interleaved G2/v/proj ----
            with (
                tc.tile_pool(name="ps", bufs=3, space="PSUM") as ps,
                tc.tile_pool(name="ps_po", bufs=2, space="PSUM") as ps_po,
                tc.tile_pool(name="p8_pool", bufs=20) as p8_pool,
                tc.tile_pool(name="ot_pool", bufs=12) as ot_pool,
                tc.tile_pool(name="osb_pool", bufs=3) as osb_pool,
                tc.tile_pool(name="prt_pool", bufs=4) as prt_pool,
                tc.tile_pool(name="out_pool", bufs=4) as out_pool,
                tc.tile_pool(name="nrm_pool", bufs=1) as nrm_pool,
            ):
                state = {}   # per-unit live tiles: stage/p8/po

                def emit_v_chunk(c):
                    """v for token chunk c -> v8[c//2] half (fp8 + ones)."""
                    j, ko = c // 2, c % 2
                    if ko == 0:
                        nc.vector.memset(v8[j][:], 1.0)
                    pv = ps.tile([P, 1024], F32, name="ps")
                    for k in range(DC):
                        nc.tensor.matmul(
                            pv[:, 0:OF], xt_t[k][:, c * P:(c + 1) * P],
                            wv_s(k), start=(k == 0), stop=(k == DC - 1))
                    for h in range(HPC):
                        nc.vector.tensor_add(
                            v8[j][:, ko * VKO + h * VH:ko * VKO + h * VH + DH],
                            pv[:, h * DH:(h + 1) * DH],
                            bv_bc[:, h * DH:(h + 1) * DH])

                def emit_g2_group(g):
                    """qkT for m in {1,3}: group g = (m, n) with k-inner."""
                    m = 1 if g < QT else 3
                    n = g % QT
                    pg = ps.tile([P, 1024], F32, name="ps")
                    for k in range(DC):
                        nc.tensor.matmul(
                            pg[:, 0:NT], wqk_s(k, m),
                            xt_t[k][:, n * NT:(n + 1) * NT],
                            start=(k == 0), stop=(k == DC - 1))
                    nc.vector.tensor_scalar_add(
                        qk_sb[m][:, n * NT:(n + 1) * NT], pg[:, 0:NT],
                        bqk_sb[:, m:m + 1])

                def emit_scores(u, j):
                    """Paired-row score MMs for unit u, chunk pair j; exp
                    straight from psum -> fp8 p tiles (PV-ready layout)."""
                    p, qt = u
                    kt = qk_sb[2 + p]
                    qt_ = qk_sb[p]
                    qs = slice(qt * NT, (qt + 1) * NT)
                    st = state[u]
                    swA = ps.tile([P, 1024], F32, name="ps")
                    swB = ps.tile([P, 1024], F32, name="ps")
                    for ko in range(2):
                        c = 2 * j + ko
                        cs = slice(c * P, (c + 1) * P)
                        nc.tensor.matmul(swA[:, ko * NT:(ko + 1) * NT],
                                         kt[0:64, cs], qt_[0:64, qs],
                                         tile_position=(0, 0),
                                         start=True, stop=True)
                        nc.tensor.matmul(swB[:, ko * NT:(ko + 1) * NT],
                                         kt[64:128, cs], qt_[64:128, qs],
                                         tile_position=(64, 0),
                                         start=True, stop=True)
                    for hd, sw in ((0, swA), (1, swB)):
                        p8t = p8_pool.tile([P, 1024], FP8, name="p8")
                        nc.scalar.activation(p8t[:], sw[:],
                                             mybir.ActivationFunctionType.Exp)
                        st["p8"][(hd, j)] = p8t

                def emit_pv(u, j):
                    st = state[u]
                    if j == 0:
                        # allocate po lazily: prev unit's po tiles are
                        # released (normalize) before this executes
                        st["po"] = [ps_po.tile([DH + 1, NT], F32, name="po")
                                    for _ in range(2)]
                    for hd in range(2):
                        p8t = st["p8"][(hd, j)]
                        rhs = p8t[:, :].rearrange("p (ko n) -> p ko n", ko=2)
                        lhs = v8[j][:, :].rearrange("p (ko x) -> p ko x", ko=2)
                        p_, qt_i = u
                        h = 2 * p_ + hd  # local head within the 4-head group
                        lhs = lhs[:, :, h * VH:h * VH + DH + 1]
                        nc.tensor.matmul(
                            st["po"][hd][:], lhs, rhs,
                            perf_mode=mybir.MatmulPerfMode.DoubleRow,
                            start=(j == 0), stop=(j == KC // 2 - 1))

                ag_fired = set()

                def emit_normalize(u):
                    p, qt = u
                    st = state[u]
                    o_sb = osb_pool.tile([P, NT], BF16, name="osb")
                    for hd in range(2):
                        po = st["po"][hd]
                        dn = nrm_pool.tile([1, NT], F32, name=f"dn{hd}")
                        nc.vector.tensor_copy(dn[0:1, :], po[64:65, :])
                        rc = nrm_pool.tile([1, NT], F32, name=f"rc{hd}")
                        scr = nrm_pool.tile([1, NT], F32, name=f"scr{hd}")
                        nc.vector.reciprocal_approx_accurate(
                            rc[0:1, :], dn[0:1, :], scr[0:1, :])
                        rb = nrm_pool.tile([64, NT], F32, name=f"rb{hd}")
                        nc.gpsimd.partition_broadcast(rb[0:64, :], rc[0:1, :])
                        nc.vector.tensor_mul(
                            o_sb[hd * 64:(hd + 1) * 64, :], po[0:64, :],
                            rb[0:64, :])
                    if qt == 3:
                        nc.sync.dma_start(ag_in[(3, p)][:, :], o_sb[:])
                        nc.gpsimd.collective_compute(
                            "AllGather", mybir.AluOpType.bypass,
                            replica_groups=GROUPS,
                            ins=[ag_in[(3, p)].ap().opt()],
                            outs=[ag_out[(3, p)].ap().opt()])
                        return
                    nc.sync.dma_start(ag_in[qt][p * P:(p + 1) * P, :], o_sb[:])
                    # one AllGather per qtile, fired once both pairs landed
                    if qt in ag_fired:
                        nc.gpsimd.collective_compute(
                            "AllGather", mybir.AluOpType.bypass,
                            replica_groups=GROUPS,
                            ins=[ag_in[qt].ap().opt()],
                            outs=[ag_out[qt].ap().opt()])
                    ag_fired.add(qt)

                def emit_proj(qt):
                    """Reload gathered oT chunks for qtile and project."""
                    ots = {}
                    for r in range(4):
                        for p in range(2):
                            t = ot_pool.tile([P, NT], BF16, name="ot")
                            if qt == 3:
                                src = ag_out[(3, p)][r * P:(r + 1) * P, :]
                            else:
                                src = ag_out[qt][r * 2 * P + p * P:
                                                 r * 2 * P + (p + 1) * P, :]
                            nc.sync.dma_start(t[:], src)
                            ots[(r, p)] = t
                    for f in range(2):
                        ppt = ps.tile([P, 1024], F32, name="ps")
                        pp = ppt[:, 0:NT]
                        for r in range(4):
                            for p in range(2):
                                nc.tensor.matmul(
                                    pp, wp_s(2 * r + p, f), ots[(r, p)][:],
                                    start=(r == 0 and p == 0),
                                    stop=(r == 3 and p == 1))
                        ou = out_pool.tile([P, NT], F32, name="ou")
                        nc.vector.tensor_scalar_add(
                            ou[:], pp, bp_sb[:, f:f + 1])
                        for q4 in range(4):
                            nc.sync.dma_start(
                                outT_ext[f * P + q4 * 32:f * P + (q4 + 1) * 32,
                                         qt * NT:(qt + 1) * NT],
                                ou[q4 * 32:(q4 + 1) * 32, :])

                def emit_proj3_half(p):
                    """qt3 projection: p0 partial early, p1 final late."""
                    ots = []
                    for r in range(4):
                        t = ot_pool.tile([P, NT], BF16, name="ot")
                        nc.sync.dma_start(t[:], ag_out[(3, p)][r * P:(r + 1) * P, :])
                        ots.append(t)
                    for f in range(2):
                        ppt = ps.tile([P, 1024], F32, name="ps")
                        pp = ppt[:, 0:NT]
                        for r in range(4):
                            nc.tensor.matmul(
                                pp, wp_s(2 * r + p, f), ots[r][:],
                                start=(r == 0), stop=(r == 3))
                        if p == 0:
                            prt = prt_pool.tile([P, NT], F32, name=f"prt{f}")
                            nc.vector.tensor_copy(prt[:], pp)
                            proj_partial[f] = prt
                        else:
                            ou = out_pool.tile([P, NT], F32, name="ou")
                            nc.vector.scalar_tensor_tensor(
                                ou[:], pp, bp_sb[:, f:f + 1],
                                proj_partial[f][:],
                                op0=mybir.AluOpType.add,
                                op1=mybir.AluOpType.add)
                            for q4 in range(4):
                                nc.sync.dma_start(
                                    outT_ext[f * P + q4 * 32:
                                             f * P + (q4 + 1) * 32,
                                             3 * NT:4 * NT],
                                    ou[q4 * 32:(q4 + 1) * 32, :])
                proj_partial = {}

                # projection late (never blocks the PE queue on a pending
                # collective); qt3 split into halves: "3a" = p0 partial
                # (its gather fired back at slot 5), "3b" = p1 final
                proj_at = {8: [0, 1, "3a"], 9: [2, "3b"]}

                for slot in range(10):
                    cur = UNITS[slot] if slot < 8 else None
                    prev = UNITS[slot - 1] if 1 <= slot <= 8 else None
                    if cur is not None:
                        state[cur] = {"p8": {}}
                    for j in range(KC // 2):
                        if cur is not None:
                            # scores (and their exps) first: keep ScalarE fed;
                            # v/qkT-G2 matmuls then run under the exp shadow
                            emit_scores(cur, j)
                            if slot in (0, 1):
                                emit_v_chunk(slot * 8 + j)
                            elif slot == 2:
                                emit_g2_group(j)
                        if prev is not None:
                            emit_pv(prev, j)
                    if prev is not None:
                        emit_normalize(prev)
                        del state[prev]
                    for qt in proj_at.get(slot, []):
                        if qt == "3a":
                            emit_proj3_half(0)
                        elif qt == "3b":
                            emit_proj3_half(1)
                        else:
                            emit_proj(qt)

    nc.compile()
    return nc


_NC_CACHE = None


def _get_nc():
    global _NC_CACHE
    if _NC_CACHE is None:
        _NC_CACHE = build_nc()
    return _NC_CACHE


def _bf16(a):
    return np.ascontiguousarray(a.astype(ml_dtypes.bfloat16))


def _chunked(w):  # [1024, C] -> [128, 8*C] (d_model chunk-major lines)
    C = w.shape[1]
    return w.reshape(DC, P, C).transpose(1, 0, 2).reshape(P, DC * C)


def kernel(x, w_qkv, b_qkv, w_proj, b_proj):
    global LAST_RESULTS
    x = np.asarray(x, dtype=np.float32)
    w_qkv = np.asarray(w_qkv, dtype=np.float32)
    b_qkv = np.asarray(b_qkv, dtype=np.float32)
    w_proj = np.asarray(w_proj, dtype=np.float32)
    b_proj = np.asarray(b_proj, dtype=np.float32)

    nc = _get_nc()

    in_maps = []
    for c in CORE_IDS:
        b, g = c // 4, c % 4
        cs = slice(g * OF, (g + 1) * OF)
        wq = w_qkv[:, 0 * D:1 * D][:, cs] * SCALE
        wk = w_qkv[:, 1 * D:2 * D][:, cs]
        wv = w_qkv[:, 2 * D:3 * D][:, cs]
        bq = b_qkv[0 * D:1 * D][cs] * SCALE
        bk = b_qkv[1 * D:2 * D][cs]
        bqk = np.concatenate([bq, bk]).reshape(4, P).T.copy()
        wqkp = _chunked(np.concatenate([wq, wk], axis=1))
        wvpp = np.concatenate(
            [_chunked(wv), _chunked(w_proj[:, cs])], axis=1)
        in_maps.append({
            "xt": _bf16(x[b].T),
            "wqkp": _bf16(wqkp),
            "wvpp": _bf16(wvpp),
            "bqk": np.ascontiguousarray(bqk, dtype=np.float32),
            "bv": np.ascontiguousarray(
                b_qkv[2 * D + g * OF:2 * D + (g + 1) * OF].reshape(1, OF)),
            "bp": np.ascontiguousarray(
                b_proj[cs].reshape(2, P).T, dtype=np.float32),
        })

    trace = bool(os.environ.get("KERNEL_TRACE"))
    if trace:
        _install_ntff_shim()
    LAST_RESULTS = run_bass_kernel_spmd(
        nc, in_maps, CORE_IDS, trace=trace)

    out = np.empty((B, N, D), dtype=np.float32)
    for c in CORE_IDS:
        b, g = c // 4, c % 4
        out[b, :, g * OF:(g + 1) * OF] = LAST_RESULTS.results[c]["outT"].T
    return out


# revision 33
# speedup vs baseline: 1.1912x; 1.1912x over previous
"""Distributed multi-head attention kernel for one TRN2 chip (8 NeuronCores).

Problem: x[2,2048,1024] -> qkv -> 16-head attention -> out proj, f32 I/O.

Sharding: 8 cores = 2 batches x 4 head-groups (4 heads each).
Core c: batch b=c//4, head group g=c%4 (heads 4g..4g+3).

Design (HW-measured building blocks):
 - Score matmuls (K=64) issued on ALTERNATING PE row-groups (0,0)/(64,0):
   disjoint array halves stream concurrently (131ns/MM vs 426 serial).
 - PV matmuls fp8e4 DoubleRow: two k-chunks per MM (2x), ones column
   still emits softmax denominators in psum row 64.
 - Softmax exp on ScalarE straight from PSUM, [128,1024] per head and
   chunk-pair; output is the fp8 PV moving operand (layout-compatible).
 - Everything is emitted as one software-pipelined stream: 8 attention
   "units" (pair x qtile); each unit's PV trails one slot behind its
   scores/exp; qkT (G1 pair-0, G2 pair-1) and v production are
   interleaved at j-granularity just-in-time against the input DMA,
   which streams x in token-block-major 8KB lines.
 - Per-qtile AllGathers (qt3 split per pair), projection at the end
   (flipped: outT = wp.T @ oT, N=512), f32 output written transposed,
   host transposes back.
"""

import os
import sys
import types
import numpy as np
import ml_dtypes

import concourse.bass as bass
import concourse.mybir as mybir
import concourse.bacc as bacc
import concourse.tile as tile
from concourse.bass_utils import run_bass_kernel_spmd

BF16 = mybir.dt.bfloat16
F32 = mybir.dt.float32
FP8 = mybir.dt.float8e4

B, N, D = 2, 2048, 1024
H, DH = 16, 64
SCALE = DH ** -0.5

P = 128                  # partitions
NT = 512                 # token tile (qtile width, matmul N)
KC = N // P              # 16 k-token chunks
QT = N // NT             # 4 q tiles
DC = D // P              # 8 d_model chunks
HPC = 4                  # heads per core
OF = HPC * DH            # 256 o-features per core
VH = 68                  # v8 per-head stride: [v(64) | ones | pad3]
VKO = HPC * VH           # 272 (ko stride in v8, %16==0 for DoubleRow)

CORE_IDS = list(range(8))
GROUPS = [[0, 1, 2, 3], [4, 5, 6, 7]]
# unit order: pair-0 early (pair-1 qkT streams in under slots 1-3),
# pairs of each qtile interleaved so the per-qt AllGathers fire at
# slots 3,5,7 and only qt3's pair-1 gather is tail-exposed.
UNITS = [(0, 0), (0, 1), (1, 0), (0, 2), (1, 1), (0, 3), (1, 2), (1, 3)]
LAST_RESULTS = None


def _install_ntff_shim():
    if "antenv.axon_hooks" in sys.modules:
        return
    try:
        from trn_agent_boot.trn_boot import _ntff_profile_via_ctypes
        hook = _ntff_profile_via_ctypes("/opt/axon/libaxon_pjrt.so")
    except Exception:
        hook = None
    mod = types.ModuleType("antenv.axon_hooks")
    mod._hook = hook
    mod.get_axon_ntff_profile_hook = lambda: mod._hook
    mod.set_axon_ntff_profile_hook = lambda h: setattr(mod, "_hook", h)
    sys.modules["antenv.axon_hooks"] = mod


def build_nc():
    nc = bacc.Bacc("TRN2", target_bir_lowering=False, debug=False, num_devices=8)

    # x in token-block-major chunk-packed lines (8KB per partition)
    xtp_ext = [nc.dram_tensor(f"xtp{n}", [P, DC * NT], BF16,
                              kind="ExternalInput") for n in range(QT)]
    wqkA_ext = nc.dram_tensor("wqkA", [P, 2048], BF16, kind="ExternalInput")
    wqkB_ext = nc.dram_tensor("wqkB", [P, 2048], BF16, kind="ExternalInput")
    wvp_ext = nc.dram_tensor("wvp", [P, 2048], BF16, kind="ExternalInput")
    wpp_ext = nc.dram_tensor("wpp", [P, 2048], BF16, kind="ExternalInput")
    bqk_ext = nc.dram_tensor("bqk", [P, 4], F32, kind="ExternalInput")
    bv_ext = nc.dram_tensor("bv", [1, OF], F32, kind="ExternalInput")
    bp_ext = nc.dram_tensor("bp", [P, 2], F32, kind="ExternalInput")
    outT_ext = nc.dram_tensor("outT", [OF, N], F32, kind="ExternalOutput")

    ag_in = {}
    ag_out = {}
    for qt in range(3):
        ag_in[qt] = nc.dram_tensor(f"agi{qt}", [2 * P, NT], BF16)
        ag_out[qt] = nc.dram_tensor(f"ago{qt}", [8 * P, NT], BF16)
    for p in range(2):  # qt3 split per pair so only the p1 gather is exposed
        ag_in[(3, p)] = nc.dram_tensor(f"agi3{p}", [P, NT], BF16)
        ag_out[(3, p)] = nc.dram_tensor(f"ago3{p}", [4 * P, NT], BF16)

    with tile.TileContext(nc) as tc:
        with (
            tc.tile_pool(name="const_pool", bufs=1) as const_pool,
            tc.tile_pool(name="xt_pool", bufs=1) as xt_pool,
            tc.tile_pool(name="w_pool", bufs=1) as w_pool,
            tc.tile_pool(name="qk_pool", bufs=1) as qk_pool,
            tc.tile_pool(name="v8_pool", bufs=1) as v8_pool,
        ):
            # ---- tiny tensors first (unblock ACT + evictions early) ----
            bqk_sb = const_pool.tile([P, 4], F32)
            nc.sync.dma_start(bqk_sb[:], bqk_ext[:])
            bv_row = const_pool.tile([1, OF], F32)
            nc.sync.dma_start(bv_row[:], bv_ext[:])
            bp_sb = const_pool.tile([P, 2], F32)
            nc.sync.dma_start(bp_sb[:], bp_ext[:])

            # ACT table warm-up: get the Exp ACT_TABLE_LOAD done at t=0
            wu_in = const_pool.tile([P, 16], BF16)
            nc.vector.memset(wu_in[:], 0.0)
            wu_out = const_pool.tile([P, 16], FP8)
            nc.scalar.activation(wu_out[:], wu_in[:],
                                 mybir.ActivationFunctionType.Exp)

            bv_bc = const_pool.tile([P, OF], F32)
            nc.gpsimd.partition_broadcast(bv_bc[:], bv_row[:])

            # ---- bulk input DMA: ring-split, ordered by first use ----
            wqkA = w_pool.tile([P, 2048], BF16, name="wqkA")
            wqkB = w_pool.tile([P, 2048], BF16, name="wqkB")
            wvp = w_pool.tile([P, 2048], BF16, name="wvp")
            wpp = w_pool.tile([P, 2048], BF16, name="wpp")
            xtn = [xt_pool.tile([P, DC * NT], BF16, name=f"xt{n}")
                   for n in range(QT)]

            def dma4(dst, src, splits=4):
                step = P // splits
                for q in range(splits):
                    nc.sync.dma_start(dst[q * step:(q + 1) * step, :],
                                      src[q * step:(q + 1) * step, :])

            dma4(wqkA[:, :], wqkA_ext[:, :])
            for n in range(QT):
                dma4(xtn[n][:, :], xtp_ext[n][:, :], splits=8)
            dma4(wvp[:, :], wvp_ext[:, :])
            dma4(wqkB[:, :], wqkB_ext[:, :])
            dma4(wpp[:, :], wpp_ext[:, :])

            def wqk_s(k, m):
                t = wqkA if m in (0, 2) else wqkB
                return t[:, (m // 2) * 1024 + k * P:(m // 2) * 1024 + (k + 1) * P]

            def wv_s(k):
                return wvp[:, k * OF:(k + 1) * OF]

            def wp_s(kk, f):
                return wpp[:, kk * OF + f * P:kk * OF + (f + 1) * P]

            def xt_r(k, n):  # qkT moving: chunk-k features x token block n
                return xtn[n][:, k * NT:(k + 1) * NT]

            def xt_v(k, c):  # v stationary: chunk-k features x token chunk c
                n, o = c // 4, (c % 4) * P
                return xtn[n][:, k * NT + o:k * NT + o + P]

            qk_sb = [qk_pool.tile([P, N], BF16, name=f"qk{m}") for m in range(4)]
            v8 = [v8_pool.tile([P, 2 * VKO], FP8, name=f"v8{j}")
                  for j in range(KC // 2)]

            with (
                tc.tile_pool(name="ps", bufs=3, space="PSUM") as ps,
                tc.tile_pool(name="ps_po", bufs=2, space="PSUM") as ps_po,
                tc.tile_pool(name="p8_pool", bufs=20) as p8_pool,
                tc.tile_pool(name="ot_pool", bufs=12) as ot_pool,
                tc.tile_pool(name="osb_pool", bufs=3) as osb_pool,
                tc.tile_pool(name="prt_pool", bufs=2) as prt_pool,
                tc.tile_pool(name="out_pool", bufs=4) as out_pool,
                tc.tile_pool(name="nrm_pool", bufs=1) as nrm_pool,
            ):
                state = {}

                def emit_qkt_group(m, n):
                    """qkT for one (m, n): 8 accumulating MMs + bias evict."""
                    pg = ps.tile([P, 1024], F32, name="ps")
                    for k in range(DC):
                        nc.tensor.matmul(
                            pg[:, 0:NT], wqk_s(k, m), xt_r(k, n),
                            start=(k == 0), stop=(k == DC - 1))
                    nc.vector.tensor_scalar_add(
                        qk_sb[m][:, n * NT:(n + 1) * NT], pg[:, 0:NT],
                        bqk_sb[:, m:m + 1])

                def emit_v_pair(j):
                    """v for chunks 2j, 2j+1 -> v8[j] (fp8, ones cols)."""
                    nc.vector.memset(v8[j][:], 1.0)
                    pv = ps.tile([P, 1024], F32, name="ps")
                    for ko in range(2):
                        c = 2 * j + ko
                        for k in range(DC):
                            nc.tensor.matmul(
                                pv[:, ko * NT:ko * NT + OF], xt_v(k, c),
                                wv_s(k), start=(k == 0), stop=(k == DC - 1))
                    for ko in range(2):
                        for h in range(HPC):
                            nc.vector.tensor_add(
                                v8[j][:, ko * VKO + h * VH:ko * VKO + h * VH + DH],
                                pv[:, ko * NT + h * DH:ko * NT + (h + 1) * DH],
                                bv_bc[:, h * DH:(h + 1) * DH])

                def emit_scores(u, j):
                    """Paired-row score MMs; exp psum -> fp8 p tiles."""
                    p, qt = u
                    kt = qk_sb[2 + p]
                    qt_ = qk_sb[p]
                    qs = slice(qt * NT, (qt + 1) * NT)
                    st = state[u]
                    swA = ps.tile([P, 1024], F32, name="ps")
                    swB = ps.tile([P, 1024], F32, name="ps")
                    for ko in range(2):
                        c = 2 * j + ko
                        cs = slice(c * P, (c + 1) * P)
                        nc.tensor.matmul(swA[:, ko * NT:(ko + 1) * NT],
                                         kt[0:64, cs], qt_[0:64, qs],
                                         tile_position=(0, 0),
                                         start=True, stop=True)
                        nc.tensor.matmul(swB[:, ko * NT:(ko + 1) * NT],
                                         kt[64:128, cs], qt_[64:128, qs],
                                         tile_position=(64, 0),
                                         start=True, stop=True)
                    for hd, sw in ((0, swA), (1, swB)):
                        p8t = p8_pool.tile([P, 1024], FP8, name="p8")
                        nc.scalar.activation(p8t[:], sw[:],
                                             mybir.ActivationFunctionType.Exp)
                        st["p8"][(hd, j)] = p8t

                def emit_pv(u, j):
                    st = state[u]
                    if j == 0:
                        st["po"] = [ps_po.tile([DH + 1, NT], F32, name="po")
                                    for _ in range(2)]
                    for hd in range(2):
                        p8t = st["p8"][(hd, j)]
                        rhs = p8t[:, :].rearrange("p (ko n) -> p ko n", ko=2)
                        lhs = v8[j][:, :].rearrange("p (ko x) -> p ko x", ko=2)
                        p_, qt_i = u
                        h = 2 * p_ + hd
                        lhs = lhs[:, :, h * VH:h * VH + DH + 1]
                        nc.tensor.matmul(
                            st["po"][hd][:], lhs, rhs,
                            perf_mode=mybir.MatmulPerfMode.DoubleRow,
                            start=(j == 0), stop=(j == KC // 2 - 1))

                ag_fired = set()

                def emit_normalize(u):
                    p, qt = u
                    st = state[u]
                    o_sb = osb_pool.tile([P, NT], BF16, name="osb")
                    for hd in range(2):
                        po = st["po"][hd]
                        dn = nrm_pool.tile([1, NT], F32, name=f"dn{hd}")
                        nc.vector.tensor_copy(dn[0:1, :], po[64:65, :])
                        rc = nrm_pool.tile([1, NT], F32, name=f"rc{hd}")
                        scr = nrm_pool.tile([1, NT], F32, name=f"scr{hd}")
                        nc.vector.reciprocal_approx_accurate(
                            rc[0:1, :], dn[0:1, :], scr[0:1, :])
                        rb = nrm_pool.tile([64, NT], F32, name=f"rb{hd}")
                        nc.gpsimd.partition_broadcast(rb[0:64, :], rc[0:1, :])
                        nc.vector.tensor_mul(
                            o_sb[hd * 64:(hd + 1) * 64, :], po[0:64, :],
                            rb[0:64, :])
                    if qt == 3:
                        nc.sync.dma_start(ag_in[(3, p)][:, :], o_sb[:])
                        nc.gpsimd.collective_compute(
                            "AllGather", mybir.AluOpType.bypass,
                            replica_groups=GROUPS,
                            ins=[ag_in[(3, p)].ap().opt()],
                            outs=[ag_out[(3, p)].ap().opt()])
                        return
                    nc.sync.dma_start(ag_in[qt][p * P:(p + 1) * P, :], o_sb[:])
                    if qt in ag_fired:
                        nc.gpsimd.collective_compute(
                            "AllGather", mybir.AluOpType.bypass,
                            replica_groups=GROUPS,
                            ins=[ag_in[qt].ap().opt()],
                            outs=[ag_out[qt].ap().opt()])
                    ag_fired.add(qt)

                def emit_out_dma(f, qt, ou):
                    for q4 in range(4):
                        nc.sync.dma_start(
                            outT_ext[f * P + q4 * 32:f * P + (q4 + 1) * 32,
                                     qt * NT:(qt + 1) * NT],
                            ou[q4 * 32:(q4 + 1) * 32, :])

                def emit_proj(qt):
                    ots = {}
                    for r in range(4):
                        for p in range(2):
                            t = ot_pool.tile([P, NT], BF16, name="ot")
                            nc.sync.dma_start(
                                t[:], ag_out[qt][r * 2 * P + p * P:
                                                 r * 2 * P + (p + 1) * P, :])
                            ots[(r, p)] = t
                    for f in range(2):
                        ppt = ps.tile([P, 1024], F32, name="ps")
                        pp = ppt[:, 0:NT]
                        for r in range(4):
                            for p in range(2):
                                nc.tensor.matmul(
                                    pp, wp_s(2 * r + p, f), ots[(r, p)][:],
                                    start=(r == 0 and p == 0),
                                    stop=(r == 3 and p == 1))
                        ou = out_pool.tile([P, NT], F32, name="ou")
                        nc.vector.tensor_scalar_add(ou[:], pp, bp_sb[:, f:f + 1])
                        emit_out_dma(f, qt, ou[:])

                proj_partial = {}

                def emit_proj3_half(p):
                    ots = []
                    for r in range(4):
                        t = ot_pool.tile([P, NT], BF16, name="ot")
                        nc.sync.dma_start(
                            t[:], ag_out[(3, p)][r * P:(r + 1) * P, :])
                        ots.append(t)
                    for f in range(2):
                        ppt = ps.tile([P, 1024], F32, name="ps")
                        pp = ppt[:, 0:NT]
                        for r in range(4):
                            nc.tensor.matmul(
                                pp, wp_s(2 * r + p, f), ots[r][:],
                                start=(r == 0), stop=(r == 3))
                        if p == 0:
                            prt = prt_pool.tile([P, NT], F32, name=f"prt{f}")
                            nc.vector.tensor_copy(prt[:], pp)
                            proj_partial[f] = prt
                        else:
                            ou = out_pool.tile([P, NT], F32, name="ou")
                            nc.vector.scalar_tensor_tensor(
                                ou[:], pp, bp_sb[:, f:f + 1],
                                proj_partial[f][:],
                                op0=mybir.AluOpType.add,
                                op1=mybir.AluOpType.add)
                            emit_out_dma(f, 3, ou[:])

                # just-in-time extras: (slot, j) -> emit thunks.  Deadlines:
                # qkT m2-block n feeds scores(0,*) at j=2n..2n+1; m3-block n
                # feeds scores(1,*) likewise; m1-block n feeds unit (1,n);
                # v8[j] feeds PV(U0, j) during slot 1.
                EX = {}
                EX[(0, 0)] = [lambda: emit_qkt_group(0, 1)]
                EX[(0, 1)] = [lambda: emit_qkt_group(2, 1)]
                EX[(0, 2)] = [lambda: emit_qkt_group(0, 2)]
                EX[(0, 3)] = [lambda: emit_qkt_group(2, 2)]
                EX[(0, 4)] = [lambda: emit_qkt_group(0, 3)]
                EX[(0, 5)] = [lambda: emit_qkt_group(2, 3)]
                EX[(0, 6)] = [lambda: emit_v_pair(0)]
                EX[(0, 7)] = [lambda: emit_v_pair(1)]
                for jj in range(6):
                    EX[(1, jj)] = [lambda j2=jj + 2: emit_v_pair(j2)]
                EX[(1, 6)] = [lambda: emit_qkt_group(3, 0)]
                EX[(1, 7)] = [lambda: emit_qkt_group(1, 0)]
                EX[(2, 0)] = [lambda: emit_qkt_group(3, 1)]
                EX[(2, 2)] = [lambda: emit_qkt_group(3, 2)]
                EX[(2, 4)] = [lambda: emit_qkt_group(3, 3)]
                EX[(2, 6)] = [lambda: emit_qkt_group(1, 1)]
                EX[(3, 0)] = [lambda: emit_qkt_group(1, 2)]
                EX[(3, 2)] = [lambda: emit_qkt_group(1, 3)]

                proj_at = {8: [0, 1, "3a"], 9: [2, "3b"]}

                # G1 for the first unit's q/k columns
                emit_qkt_group(0, 0)
                emit_qkt_group(2, 0)

                for slot in range(10):
                    cur = UNITS[slot] if slot < 8 else None
                    prev = UNITS[slot - 1] if 1 <= slot <= 8 else None
                    if cur is not None:
                        state[cur] = {"p8": {}}
                    for j in range(KC // 2):
                        if cur is not None:
                            emit_scores(cur, j)
                            for fn in EX.get((slot, j), []):
                                fn()
                        if prev is not None:
                            emit_pv(prev, j)
                    if prev is not None:
                        emit_normalize(prev)
                        del state[prev]
                    for qt in proj_at.get(slot, []):
                        if qt == "3a":
                            emit_proj3_half(0)
                        elif qt == "3b":
                            emit_proj3_half(1)
                        else:
                            emit_proj(qt)

    nc.compile()
    return nc


_NC_CACHE = None


def _get_nc():
    global _NC_CACHE
    if _NC_CACHE is None:
        _NC_CACHE = build_nc()
    return _NC_CACHE


def _bf16(a):
    return np.ascontiguousarray(a.astype(ml_dtypes.bfloat16))


def _chunked(w):  # [1024, C] -> [128, 8*C] (d_model-chunk-major lines)
    C = w.shape[1]
    return w.reshape(DC, P, C).transpose(1, 0, 2).reshape(P, DC * C)


def kernel(x, w_qkv, b_qkv, w_proj, b_proj):
    global LAST_RESULTS
    x = np.asarray(x, dtype=np.float32)
    w_qkv = np.asarray(w_qkv, dtype=np.float32)
    b_qkv = np.asarray(b_qkv, dtype=np.float32)
    w_proj = np.asarray(w_proj, dtype=np.float32)
    b_proj = np.asarray(b_proj, dtype=np.float32)

    nc = _get_nc()

    in_maps = []
    for c in CORE_IDS:
        b, g = c // 4, c % 4
        cs = slice(g * OF, (g + 1) * OF)
        wq = w_qkv[:, 0 * D:1 * D][:, cs] * SCALE
        wk = w_qkv[:, 1 * D:2 * D][:, cs]
        wv = w_qkv[:, 2 * D:3 * D][:, cs]
        bq = b_qkv[0 * D:1 * D][cs] * SCALE
        bk = b_qkv[1 * D:2 * D][cs]
        bqk = np.concatenate([bq, bk]).reshape(4, P).T.copy()
        im = {
            "wqkA": _bf16(np.concatenate(
                [_chunked(wq[:, 0:P]), _chunked(wk[:, 0:P])], axis=1)),
            "wqkB": _bf16(np.concatenate(
                [_chunked(wq[:, P:OF]), _chunked(wk[:, P:OF])], axis=1)),
            "wvp": _bf16(_chunked(wv)),
            "wpp": _bf16(_chunked(w_proj[:, cs])),
            "bqk": np.ascontiguousarray(bqk, dtype=np.float32),
            "bv": np.ascontiguousarray(
                b_qkv[2 * D + g * OF:2 * D + (g + 1) * OF].reshape(1, OF)),
            "bp": np.ascontiguousarray(
                b_proj[cs].reshape(2, P).T, dtype=np.float32),
        }
        for n in range(QT):
            blockT = np.ascontiguousarray(x[b][n * NT:(n + 1) * NT, :].T)
            im[f"xtp{n}"] = _bf16(_chunked(blockT))
        in_maps.append(im)

    trace = bool(os.environ.get("KERNEL_TRACE"))
    if trace:
        _install_ntff_shim()
    LAST_RESULTS = run_bass_kernel_spmd(
        nc, in_maps, CORE_IDS, trace=trace)

    out = np.empty((B, N, D), dtype=np.float32)
    for c in CORE_IDS:
        b, g = c // 4, c % 4
        out[b, :, g * OF:(g + 1) * OF] = LAST_RESULTS.results[c]["outT"].T
    return out
